# revision 52
# baseline (speedup 1.0000x reference)
"""Multi-head attention (B=8, N=1024, C=768, H=12, D=64) on 8 TRN2 NeuronCores.

Sharding: pure data parallel - one batch element per core, weights replicated,
no collectives. Each core computes its full attention block.

fp16 operands, fp32 PSUM. The kernel is ACT-bound in steady state (96 exp
ACTIVATEs of [128,1024] ~= 107us busy), so the design keeps ACT fed and
shrinks the head/tail where ACT is idle. Shaped by per-instruction NTFF
analysis; the main mechanisms:
  - scores: the head pair's K=64 matmuls are issued adjacently with
    stationaries at base partitions 0/64, landing in disjoint PE row groups
    so the two streams run concurrently (verified dstart ~= 4ns).
  - exp: 1/sqrt(D) scale and a -2.0 bias folded into the ACTIVATE's free
    affine stage (softmax is shift-invariant), so qkT PSUM->SBUF casts are
    plain copies.
  - head: batched >=0.8MB input DMAs on 3 queues (per-tile DMAs cap at
    ~75GB/s, 1MB reaches ~340GB/s); ci-major PE transposes, 4 per PSUM bank
    with one batched copy per bank alternating DVE/ACT; a dummy accumulation
    chain trips the HAM clock gate out of its 1.2GHz idle state (transposes
    do not count as PE activity).
  - scheduling: proj partials and the softmax-normalization chains carry a
    late scheduler priority; the Tile scheduler otherwise hoists their
    matmuls (which wait on long DVE chains) ahead of latency-critical score
    matmuls, head-of-line blocking the in-order PE queue and starving ACT.
  - normalization: denominator accumulated for free as a ones-column in the
    attn@v stationary ([128,12,65] per m-tile); reciprocal_approx_fast (the
    custom DVE uop requires SBUF source and partition-0 alignment), PE
    broadcast, fused multiply into the fp16 onorm arena laid out so the
    projection reads it as [p, cpair, j, n] contraction blocks.
  - tail: the last pair runs from a fresh 6-bank PSUM pool after the scores
    pool closes (the old pool-ring dependency chained attn@v behind the norm
    DVE chain, idling the PE past the HAM window); both attn@v chunk chains
    run dense, each chunk's norm overlaps the projection rows that only need
    the other chunk; norm copies run on the then-idle Scalar engine; bias
    and the held fp16 partials fold in via identity-matmul accumulates; one
    fp32 output DMA per row tile on alternating queues.
"""

import numpy as np

B, N, C = 8, 1024, 768
H, D = 12, 64
F3 = 3 * C          # 2304
FQK = 2 * C         # 1536
SCALE = D ** -0.5   # 0.125
EXP_BIAS = -2.0     # exp(s*SCALE + EXP_BIAS); cancels in softmax
NT = N // 128       # 8 n-tiles / m-tiles
CT = C // 128       # 6 c-tiles
FT = FQK // 128     # 12 qk feature tiles
NCH = N // 512      # 2 psum chunks over n
VCH = 384           # v / proj free chunk (C = 2*384)
CP = CT // 2        # 3 c-pairs (onorm arena grouping)

_compiled = None


def _build():
    import concourse.mybir as mybir
    import concourse.tile as tile
    from concourse import bacc
    from concourse.masks import make_identity

    f32 = mybir.dt.float32
    f16 = mybir.dt.float16

    nc = bacc.Bacc("TRN2", target_bir_lowering=False, debug=False)

    x_d = nc.dram_tensor("x", [N, C], f16, kind="ExternalInput").ap()
    wqkv_d = nc.dram_tensor("w_qkv", [C, F3], f16, kind="ExternalInput").ap()
    wproj_d = nc.dram_tensor("w_proj", [C, C], f16, kind="ExternalInput").ap()
    bias_d = nc.dram_tensor("b_bcast", [128, C], f32, kind="ExternalInput").ap()
    out_d = nc.dram_tensor("out", [N, C], f32, kind="ExternalOutput").ap()

    with tile.TileContext(nc) as tc:
        with tc.tile_pool(name="const", bufs=1) as const_pool:
            ones_f32 = const_pool.tile([1, 128], f32)
            nc.gpsimd.memset(ones_f32[:], 1.0)
            sel = const_pool.tile([1, 128], f16)
            nc.vector.tensor_copy(sel[:], ones_f32[:])
            vones_f32 = const_pool.tile([128, NT * H], f32)
            nc.gpsimd.memset(vones_f32[:], 1.0)
            ident_f32 = const_pool.tile([128, 128], f32)
            make_identity(nc, ident_f32[:])
            ident = const_pool.tile([128, 128], f16)
            nc.vector.tensor_copy(ident[:], ident_f32[:])
            bias_sb = const_pool.tile([128, C], f32)
            nc.gpsimd.dma_start(bias_sb[:], bias_d)
            bias16 = const_pool.tile([128, C], f16)
            exp_bias = const_pool.tile([128, 1], f32)
            nc.gpsimd.memset(exp_bias[:], EXP_BIAS)

            # ---- persistent activations ----
            with tc.tile_pool(name="acts", bufs=1) as acts:
                xT = [acts.tile([128, N], f16, tag=f"xT{ci}", name=f"xT{ci}")
                      for ci in range(CT)]
                qkT = [acts.tile([128, N], f16, tag=f"qkT{fi}", name=f"qkT{fi}")
                       for fi in range(FT)]
                # attn@v stationary: [p, mi, h, d(+ones)], m = mi*128 + p
                vnat = acts.tile([128, NT, H, D + 1], f16, tag="vnat",
                                 name="vnat")
                # proj stationary: [p, cpair, j, n]; contraction
                # hd = cpair*256 + j*128 + p; head h lives at
                # [64*(h%2):64*(h%2)+64, h//4, (h//2)%2, :]
                onorm = acts.tile([128, CP, 2, N], f16, tag="onorm",
                                  name="onorm")

                with tc.tile_pool(name="wq", bufs=1) as wq_pool, \
                     tc.tile_pool(name="wp", bufs=1) as wp_pool, \
                     tc.tile_pool(name="xin", bufs=1) as xin_pool:
                    # ---- phase 0: batched >=0.8MB input DMAs (small
                    # per-tile DMAs cap at ~75GB/s; 1MB reaches ~340GB/s) ----
                    xt_a = xin_pool.tile([128, NT, C], f16, tag="xin",
                                         name="xin")
                    x_r = x_d.rearrange("(a p) c -> p a c", p=128)
                    nc.sync.dma_start(xt_a[:, 0:3, :], x_r[:, 0:3, :])
                    nc.gpsimd.dma_start(xt_a[:, 3:6, :], x_r[:, 3:6, :])
                    nc.scalar.dma_start(xt_a[:, 6:8, :], x_r[:, 6:8, :])
                    # dummy accumulation chain while DMAs stream: keeps the
                    # HAM clock gate from starting the kernel at 1.2GHz
                    wrm_cm = tc.tile_pool(name="wrm", bufs=1, space="PSUM")
                    wrm_pool = wrm_cm.__enter__()
                    wps = wrm_pool.tile([128, 128], f32, tag="w", name="wps")
                    for k in range(40):
                        nc.tensor.matmul(wps[:], ident[:], ident[:],
                                         start=(k == 0), stop=(k == 39))
                    wq_t = wq_pool.tile([128, CT, F3], f16, tag="wq",
                                        name="wq")
                    wq_r = wqkv_d.rearrange("(a p) f -> p a f", p=128)
                    nc.sync.dma_start(wq_t[:, 0:3, :], wq_r[:, 0:3, :])
                    nc.gpsimd.dma_start(wq_t[:, 3:6, :], wq_r[:, 3:6, :])
                    wq = [wq_t[:, ci, :] for ci in range(CT)]
                    wp_t = wp_pool.tile([128, CT, C], f16, tag="wp",
                                        name="wp")
                    wp_r = wproj_d.rearrange("(a p) c -> p a c", p=128)
                    nc.sync.dma_start(wp_t[:, 0:3, :], wp_r[:, 0:3, :])
                    nc.gpsimd.dma_start(wp_t[:, 3:6, :], wp_r[:, 3:6, :])
                    wp = [wp_t[:, ci, :] for ci in range(CT)]
                    xt_ins = [xt_a[:, ni, :] for ni in range(NT)]
                    # ci-major transposes: 4 per PSUM bank, one batched
                    # copy per bank; dummy-chain matmuls interleaved between
                    # groups keep the HAM clock gate warm (transposes do not
                    # count as PE activity), outside any accumulation group
                    with tc.tile_pool(name="ph0", bufs=2,
                                      space="PSUM") as ph0_pool:
                        for ci in range(CT):
                            for g in range(2):
                                ptg = ph0_pool.tile([128, 4, 128], f16,
                                                    tag="ptg",
                                                    name=f"ptg{ci}_{g}")
                                for k in range(4):
                                    ni = g * 4 + k
                                    nc.tensor.transpose(
                                        ptg[:, k, :],
                                        xt_ins[ni][:,
                                                   ci * 128:(ci + 1) * 128],
                                        ident[:])
                                cp = (nc.vector.tensor_copy
                                      if (ci * 2 + g) % 2 else nc.scalar.copy)
                                cp(xT[ci][:, g * 512:(g + 1) * 512],
                                   ptg[:].rearrange("p a b -> p (a b)"))
                            for k in range(3):
                                nc.tensor.matmul(wps[:], ident[:], ident[:],
                                                 start=(k == 0), stop=(k == 2))
                    wrm_cm.__exit__(None, None, None)
                    acc_cm = tc.tile_pool(name="acc", bufs=2, space="PSUM")
                    acc_pool = acc_cm.__enter__()

                    def qk_proj(fi):
                        pqk = [acc_pool.tile([128, 512], f32, tag="acc",
                                             name=f"pqk{fi}_{ch}")
                               for ch in range(NCH)]
                        for ci in range(CT):
                            for ch in range(NCH):
                                nc.tensor.matmul(
                                    pqk[ch][:],
                                    wq[ci][:, fi * 128:(fi + 1) * 128],
                                    xT[ci][:, ch * 512:(ch + 1) * 512],
                                    start=(ci == 0), stop=(ci == CT - 1))
                        for ch in range(NCH):
                            nc.vector.tensor_copy(
                                qkT[fi][:, ch * 512:(ch + 1) * 512],
                                pqk[ch][:])

                    def v_proj(ni):
                        # late priority: only attnv(0) at j==1 needs vnat;
                        # don't let these 96 matmuls crowd out pair-1 scores
                        vlp = tc.high_priority(offset=-80)
                        vlp.__enter__()
                        pv = [acc_pool.tile([128, VCH], f32, tag="acc",
                                            name=f"pv{ni}_{vc}")
                              for vc in range(2)]
                        for ci in range(CT):
                            for vc in range(2):
                                nc.tensor.matmul(
                                    pv[vc][:],
                                    xT[ci][:, ni * 128:(ni + 1) * 128],
                                    wq[ci][:, FQK + vc * VCH:
                                           FQK + (vc + 1) * VCH],
                                    start=(ci == 0), stop=(ci == CT - 1))
                        for vc in range(2):
                            nc.vector.tensor_copy(
                                vnat[:, ni, vc * 6:(vc + 1) * 6, 0:D],
                                pv[vc][:].rearrange("p (h d) -> p h d", d=D))
                        vlp.__exit__(None, None, None)

                    qk_proj(0)
                    qk_proj(6)
                    nc.vector.tensor_copy(bias16[:], bias_sb[:])

                    # ---- attention, head pairs, qk for pair j+1 interleaved
                    attn_pools = (
                        tc.tile_pool(name="fin", bufs=1),
                        tc.tile_pool(name="rc", bufs=2),
                        tc.tile_pool(name="exp", bufs=15),
                        tc.tile_pool(name="pss", bufs=2, space="PSUM"),
                        tc.tile_pool(name="pso", bufs=2, space="PSUM"),
                    )
                    fin_pool, rc_pool, exp_pool, pss_pool, pso_pool = [
                        p.__enter__() for p in attn_pools]

                    def scores_exp(j, exp_t):
                        pair = (2 * j, 2 * j + 1)
                        for mi in range(NT):
                            ps = {}
                            for h in pair:
                                ps[h] = pss_pool.tile([128, N], f32, tag="pss",
                                                      name=f"pss{h}_{mi}")
                            # adjacent e/o matmuls -> disjoint PE row groups
                            for ch in range(NCH):
                                for h in pair:
                                    qrow = (h % 2) * D
                                    nc.tensor.matmul(
                                        ps[h][:, ch * 512:(ch + 1) * 512],
                                        qkT[6 + h // 2][qrow:qrow + D,
                                                        mi * 128:(mi + 1) * 128],
                                        qkT[h // 2][qrow:qrow + D,
                                                    ch * 512:(ch + 1) * 512],
                                        start=True, stop=True)
                            for h in pair:
                                if mi % 2 == 0:
                                    et = exp_pool.tile([128, 2, N], f16,
                                                       tag="exp",
                                                       name=f"exp{h}_{mi // 2}")
                                    exp_t[h].append(et)
                                nc.scalar.activation(
                                    exp_t[h][mi // 2][:, mi % 2, :], ps[h][:],
                                    mybir.ActivationFunctionType.Exp,
                                    bias=exp_bias[:], scale=SCALE)

                    def attnv_norm(j, exp_t, po_pool, use_act):
                        pair = (2 * j, 2 * j + 1)
                        for ch in range(NCH):
                            po = {}
                            for h in pair:
                                po[h] = po_pool.tile(
                                    [D + 1, 512], f32, tag="po",
                                    name=f"po{h}_{ch}")
                            for mi in range(NT):
                                for h in pair:
                                    nc.tensor.matmul(
                                        po[h][:],
                                        vnat[:, mi, h, :],
                                        exp_t[h][mi // 2][:, mi % 2,
                                                          ch * 512:
                                                          (ch + 1) * 512],
                                        start=(mi == 0), stop=(mi == NT - 1))
                            rc = {}
                            ou = {}
                            lateprio = tc.high_priority(offset=-60)
                            lateprio.__enter__()
                            for h in pair:
                                rs = rc_pool.tile([1, 512], f32, tag="rs",
                                                  name=f"rs{h}_{ch}", bufs=2)
                                (nc.scalar.copy if use_act
                                 else nc.vector.tensor_copy)(
                                    rs[:], po[h][D:D + 1, :])
                                rcf = rc_pool.tile([1, 512], f32, tag="rcf",
                                                   name=f"rcf{h}_{ch}", bufs=2)
                                nc.vector.reciprocal_approx_fast(rcf[:], rs[:])
                                rc[h] = rc_pool.tile([1, 512], f16, tag="rc",
                                                     name=f"rc{h}_{ch}",
                                                     bufs=2)
                                (nc.scalar.copy if use_act
                                 else nc.vector.tensor_copy)(rc[h][:], rcf[:])
                                ou[h] = rc_pool.tile([D, 512], f16, tag="ou",
                                                     name=f"ou{h}_{ch}",
                                                     bufs=2)
                                if use_act:
                                    nc.scalar.copy(ou[h][:], po[h][0:D, :])
                                else:
                                    nc.vector.tensor_copy(ou[h][:],
                                                          po[h][0:D, :])
                            for h in pair:
                                # per-head broadcast of 1/den to 64 rows
                                pb = po_pool.tile([D, 512], f32, tag="po",
                                                  name=f"pb{h}_{ch}")
                                nc.tensor.matmul(
                                    pb[:], sel[0:1, 0:D], rc[h][:],
                                    start=True, stop=True)
                                s = h // 2
                                nc.vector.tensor_mul(
                                    onorm[64 * (h % 2):64 * (h % 2) + D,
                                          s // 2, s % 2,
                                          ch * 512:(ch + 1) * 512],
                                    ou[h][:], pb[0:D, :])
                            lateprio.__exit__(None, None, None)

                    finA = {}

                    def proj_a():
                        # head pairs 0-3 (cpairs 0-1) + bias via an identity
                        # matmul accumulate; keep fp16 partials in SBUF.
                        # Late scheduler priority: nothing reads finA until
                        # the tail, and hoisting these matmuls ahead of the
                        # last pairs' scores starves the exp pipeline
                        lp = tc.high_priority(offset=-80)
                        lp.__enter__()
                        for ni in range(NT):
                            pf = [acc_pool.tile([128, VCH], f32, tag="acc",
                                                name=f"pfa{ni}_{fc}")
                                  for fc in range(2)]
                            for cp in range(2):
                                for jj in range(2):
                                    for fc in range(2):
                                        nc.tensor.matmul(
                                            pf[fc][:],
                                            onorm[:, cp, jj,
                                                  ni * 128:(ni + 1) * 128],
                                            wp[cp * 2 + jj][:,
                                                fc * VCH:(fc + 1) * VCH],
                                            start=(cp == 0 and jj == 0),
                                            stop=False)
                            for fc in range(2):
                                nc.tensor.matmul(
                                    pf[fc][:], ident[:],
                                    bias16[:, fc * VCH:(fc + 1) * VCH],
                                    start=False, stop=True)
                            fa = fin_pool.tile([128, C], f16, tag="finA",
                                               name=f"finA{ni}", bufs=8)
                            finA[ni] = fa
                            for fc in range(2):
                                sl = slice(fc * VCH, (fc + 1) * VCH)
                                nc.vector.tensor_copy(fa[:, sl], pf[fc][:])
                        lp.__exit__(None, None, None)

                    def proj_b(ni, po_pool):
                        # heads 8-11 (cp2) + accumulated fp16 partial in one
                        # pass; PSUM->SBUF hops split across Scalar and DVE
                        pf = [po_pool.tile([128, VCH], f32, tag="po",
                                           name=f"pfb{ni}_{fc}")
                              for fc in range(2)]
                        for fc in range(2):
                            for jj in range(2):
                                nc.tensor.matmul(
                                    pf[fc][:],
                                    onorm[:, 2, jj, ni * 128:(ni + 1) * 128],
                                    wp[4 + jj][:, fc * VCH:(fc + 1) * VCH],
                                    start=(jj == 0), stop=False)
                            nc.tensor.matmul(
                                pf[fc][:], ident[:],
                                finA[ni][:, fc * VCH:(fc + 1) * VCH],
                                start=False, stop=True)
                        fin = fin_pool.tile([128, C], f32, tag="fin",
                                            name=f"fin{ni}", bufs=3)
                        nc.scalar.copy(fin[:, 0:VCH], pf[0][:])
                        nc.vector.tensor_copy(fin[:, VCH:C], pf[1][:])
                        eng = nc.sync if ni % 2 == 0 else nc.gpsimd
                        eng.dma_start(
                            out_d[ni * 128:(ni + 1) * 128, :], fin[:])

                    exp_ts = {}
                    for j in range(H // 2):
                        exp_ts[j] = {2 * j: [], 2 * j + 1: []}
                        scores_exp(j, exp_ts[j])
                        if j + 1 < H // 2:
                            qk_proj(j + 1)
                            qk_proj(6 + j + 1)
                        if j == 0:
                            nc.vector.tensor_copy(
                                vnat[:, :, :, D].rearrange(
                                    "p a h -> p (a h)"),
                                vones_f32[:])
                            for ni in range(NT):
                                v_proj(ni)
                        if j >= 1:
                            attnv_norm(j - 1, exp_ts.pop(j - 1), pso_pool,
                                       use_act=False)
                        if j == 4:
                            proj_a()
                    # last pair: scores PSUM banks are free now; run its
                    # attn@v from a fresh 4-slot pool so it never waits on
                    # the norm chain's pb slot recycling
                    attn_pools[4].__exit__(None, None, None)
                    attn_pools[3].__exit__(None, None, None)
                    with tc.tile_pool(name="tail", bufs=6,
                                      space="PSUM") as tail_pool:
                        # last pair: both attn@v chunk chains dense first,
                        # then per chunk: norm, then the proj rows that only
                        # need that chunk's columns (overlaps the other norm)
                        pair = (H - 2, H - 1)
                        exp_t = exp_ts.pop(H // 2 - 1)
                        po = {}
                        for ch in range(NCH):
                            for h in pair:
                                po[h, ch] = tail_pool.tile(
                                    [D + 1, 512], f32, tag="po",
                                    name=f"tpo{h}_{ch}")
                            for mi in range(NT):
                                for h in pair:
                                    nc.tensor.matmul(
                                        po[h, ch][:],
                                        vnat[:, mi, h, :],
                                        exp_t[h][mi // 2][:, mi % 2,
                                                          ch * 512:
                                                          (ch + 1) * 512],
                                        start=(mi == 0), stop=(mi == NT - 1))
                        for ch in range(NCH):
                            rc = {}
                            ou = {}
                            for h in pair:
                                rs = rc_pool.tile([1, 512], f32, tag="rs",
                                                  name=f"trs{h}_{ch}", bufs=2)
                                nc.scalar.copy(rs[:], po[h, ch][D:D + 1, :])
                                rcf = rc_pool.tile([1, 512], f32, tag="rcf",
                                                   name=f"trcf{h}_{ch}",
                                                   bufs=2)
                                nc.vector.reciprocal_approx_fast(rcf[:],
                                                                 rs[:])
                                rc[h] = rc_pool.tile([1, 512], f16, tag="rc",
                                                     name=f"trc{h}_{ch}",
                                                     bufs=2)
                                nc.scalar.copy(rc[h][:], rcf[:])
                                ou[h] = rc_pool.tile([D, 512], f16, tag="ou",
                                                     name=f"tou{h}_{ch}",
                                                     bufs=2)
                                nc.scalar.copy(ou[h][:], po[h, ch][0:D, :])
                            for h in pair:
                                pb = tail_pool.tile([D, 512], f32, tag="po",
                                                    name=f"tpb{h}_{ch}")
                                nc.tensor.matmul(
                                    pb[:], sel[0:1, 0:D], rc[h][:],
                                    start=True, stop=True)
                                s = h // 2
                                nc.vector.tensor_mul(
                                    onorm[64 * (h % 2):64 * (h % 2) + D,
                                          s // 2, s % 2,
                                          ch * 512:(ch + 1) * 512],
                                    ou[h][:], pb[0:D, :])
                            for ni in range(ch * 4, ch * 4 + 4):
                                proj_b(ni, tail_pool)

                    for p in (attn_pools[2], attn_pools[1], attn_pools[0]):
                        p.__exit__(None, None, None)
                    acc_cm.__exit__(None, None, None)

    nc.compile()
    return nc


def _get_compiled():
    global _compiled
    if _compiled is None:
        _compiled = _build()
    return _compiled


def _run(x, w_qkv, w_proj, b_proj, **kwargs):
    from concourse.bass_utils import run_bass_kernel_spmd

    x = np.asarray(x, dtype=np.float32).astype(np.float16)
    w_qkv = np.ascontiguousarray(
        np.asarray(w_qkv, dtype=np.float32).astype(np.float16))
    w_proj = np.ascontiguousarray(
        np.asarray(w_proj, dtype=np.float32).astype(np.float16))
    b_bcast = np.ascontiguousarray(
        np.broadcast_to(np.asarray(b_proj, dtype=np.float32), (128, C)))

    nc = _get_compiled()
    in_maps = [
        {"x": np.ascontiguousarray(x[b]), "w_qkv": w_qkv,
         "w_proj": w_proj, "b_bcast": b_bcast}
        for b in range(B)
    ]
    return run_bass_kernel_spmd(nc, in_maps, core_ids=list(range(B)), **kwargs)


def kernel(x, w_qkv, w_proj, b_proj, **_):
    res = _run(x, w_qkv, w_proj, b_proj)
    return np.stack([res.results[b]["out"] for b in range(B)], axis=0)


# revision 53
# speedup vs baseline: 1.0051x; 1.0051x over previous
"""Multi-head attention (B=8, N=1024, C=768, H=12, D=64) on 8 TRN2 NeuronCores.

Sharding: pure data parallel - one batch element per core, weights replicated,
no collectives. Each core computes its full attention block.

fp16 operands, fp32 PSUM. The kernel is ACT-bound in steady state (96 exp
ACTIVATEs of [128,1024] ~= 107us busy), so the design keeps ACT fed and
shrinks the head/tail where ACT is idle. Shaped by per-instruction NTFF
analysis; the main mechanisms:
  - scores: the head pair's K=64 matmuls are issued adjacently with
    stationaries at base partitions 0/64, landing in disjoint PE row groups
    so the two streams run concurrently (verified dstart ~= 4ns).
  - exp: 1/sqrt(D) scale and a -2.0 bias folded into the ACTIVATE's free
    affine stage (softmax is shift-invariant), so qkT PSUM->SBUF casts are
    plain copies.
  - head: batched >=0.8MB input DMAs on 3 queues (per-tile DMAs cap at
    ~75GB/s, 1MB reaches ~340GB/s); ci-major PE transposes, 4 per PSUM bank
    with one batched copy per bank alternating DVE/ACT; a dummy accumulation
    chain trips the HAM clock gate out of its 1.2GHz idle state (transposes
    do not count as PE activity).
  - scheduling: proj partials and the softmax-normalization chains carry a
    late scheduler priority; the Tile scheduler otherwise hoists their
    matmuls (which wait on long DVE chains) ahead of latency-critical score
    matmuls, head-of-line blocking the in-order PE queue and starving ACT.
  - normalization: denominator accumulated for free as a ones-column in the
    attn@v stationary ([128,12,65] per m-tile); reciprocal_approx_fast (the
    custom DVE uop requires SBUF source and partition-0 alignment), PE
    broadcast, fused multiply into the fp16 onorm arena laid out so the
    projection reads it as [p, cpair, j, n] contraction blocks.
  - tail: the last pair runs from a fresh 6-bank PSUM pool after the scores
    pool closes (the old pool-ring dependency chained attn@v behind the norm
    DVE chain, idling the PE past the HAM window); both attn@v chunk chains
    run dense, each chunk's norm overlaps the projection rows that only need
    the other chunk; norm copies run on the then-idle Scalar engine; bias
    and the held fp16 partials fold in via identity-matmul accumulates; one
    fp32 output DMA per row tile on alternating queues.
"""

import numpy as np

B, N, C = 8, 1024, 768
H, D = 12, 64
F3 = 3 * C          # 2304
FQK = 2 * C         # 1536
SCALE = D ** -0.5   # 0.125
EXP_BIAS = -2.0     # exp(s*SCALE + EXP_BIAS); cancels in softmax
NT = N // 128       # 8 n-tiles / m-tiles
CT = C // 128       # 6 c-tiles
FT = FQK // 128     # 12 qk feature tiles
NCH = N // 512      # 2 psum chunks over n
VCH = 384           # v / proj free chunk (C = 2*384)
CP = CT // 2        # 3 c-pairs (onorm arena grouping)

_compiled = None


def _build():
    import concourse.mybir as mybir
    import concourse.tile as tile
    from concourse import bacc
    from concourse.masks import make_identity

    f32 = mybir.dt.float32
    f16 = mybir.dt.float16

    nc = bacc.Bacc("TRN2", target_bir_lowering=False, debug=False)

    x_d = nc.dram_tensor("x", [N, C], f16, kind="ExternalInput").ap()
    wqkv_d = nc.dram_tensor("w_qkv", [C, F3], f16, kind="ExternalInput").ap()
    wproj_d = nc.dram_tensor("w_proj", [C, C], f16, kind="ExternalInput").ap()
    bias_d = nc.dram_tensor("b_bcast", [128, C], f32, kind="ExternalInput").ap()
    out_d = nc.dram_tensor("out", [N, C], f32, kind="ExternalOutput").ap()

    with tile.TileContext(nc) as tc:
        with tc.tile_pool(name="const", bufs=1) as const_pool:
            ones_f32 = const_pool.tile([1, 128], f32)
            nc.gpsimd.memset(ones_f32[:], 1.0)
            sel = const_pool.tile([1, 128], f16)
            nc.vector.tensor_copy(sel[:], ones_f32[:])
            vones_f32 = const_pool.tile([128, NT * H], f32)
            nc.gpsimd.memset(vones_f32[:], 1.0)
            ident_f32 = const_pool.tile([128, 128], f32)
            make_identity(nc, ident_f32[:])
            ident = const_pool.tile([128, 128], f16)
            nc.vector.tensor_copy(ident[:], ident_f32[:])
            bias_sb = const_pool.tile([128, C], f32)
            nc.gpsimd.dma_start(bias_sb[:], bias_d)
            bias16 = const_pool.tile([128, C], f16)
            exp_bias = const_pool.tile([128, 1], f32)
            nc.gpsimd.memset(exp_bias[:], EXP_BIAS)

            # ---- persistent activations ----
            with tc.tile_pool(name="acts", bufs=1) as acts:
                xT = [acts.tile([128, N], f16, tag=f"xT{ci}", name=f"xT{ci}")
                      for ci in range(CT)]
                qkT = [acts.tile([128, N], f16, tag=f"qkT{fi}", name=f"qkT{fi}")
                       for fi in range(FT)]
                # attn@v stationary: [p, mi, h, d(+ones)], m = mi*128 + p
                vnat = acts.tile([128, NT, H, D + 1], f16, tag="vnat",
                                 name="vnat")
                # proj stationary: [p, cpair, j, n]; contraction
                # hd = cpair*256 + j*128 + p; head h lives at
                # [64*(h%2):64*(h%2)+64, h//4, (h//2)%2, :]
                onorm = acts.tile([128, CP, 2, N], f16, tag="onorm",
                                  name="onorm")

                with tc.tile_pool(name="wq", bufs=1) as wq_pool, \
                     tc.tile_pool(name="wp", bufs=1) as wp_pool, \
                     tc.tile_pool(name="xin", bufs=1) as xin_pool:
                    # ---- phase 0: batched >=0.8MB input DMAs (small
                    # per-tile DMAs cap at ~75GB/s; 1MB reaches ~340GB/s) ----
                    xt_a = xin_pool.tile([128, NT, C], f16, tag="xin",
                                         name="xin")
                    x_r = x_d.rearrange("(a p) c -> p a c", p=128)
                    nc.sync.dma_start(xt_a[:, 0:3, :], x_r[:, 0:3, :])
                    nc.gpsimd.dma_start(xt_a[:, 3:6, :], x_r[:, 3:6, :])
                    nc.scalar.dma_start(xt_a[:, 6:8, :], x_r[:, 6:8, :])
                    # dummy accumulation chain while DMAs stream: keeps the
                    # HAM clock gate from starting the kernel at 1.2GHz
                    wrm_cm = tc.tile_pool(name="wrm", bufs=1, space="PSUM")
                    wrm_pool = wrm_cm.__enter__()
                    wps = wrm_pool.tile([128, 128], f32, tag="w", name="wps")
                    for k in range(40):
                        nc.tensor.matmul(wps[:], ident[:], ident[:],
                                         start=(k == 0), stop=(k == 39))
                    wq_t = wq_pool.tile([128, CT, F3], f16, tag="wq",
                                        name="wq")
                    wq_r = wqkv_d.rearrange("(a p) f -> p a f", p=128)
                    nc.sync.dma_start(wq_t[:, 0:3, :], wq_r[:, 0:3, :])
                    nc.gpsimd.dma_start(wq_t[:, 3:6, :], wq_r[:, 3:6, :])
                    wq = [wq_t[:, ci, :] for ci in range(CT)]
                    wp_t = wp_pool.tile([128, CT, C], f16, tag="wp",
                                        name="wp")
                    wp_r = wproj_d.rearrange("(a p) c -> p a c", p=128)
                    nc.sync.dma_start(wp_t[:, 0:3, :], wp_r[:, 0:3, :])
                    nc.gpsimd.dma_start(wp_t[:, 3:6, :], wp_r[:, 3:6, :])
                    wp = [wp_t[:, ci, :] for ci in range(CT)]
                    xt_ins = [xt_a[:, ni, :] for ni in range(NT)]
                    # ci-major transposes: 4 per PSUM bank, one batched
                    # copy per bank; dummy-chain matmuls interleaved between
                    # groups keep the HAM clock gate warm (transposes do not
                    # count as PE activity), outside any accumulation group
                    with tc.tile_pool(name="ph0", bufs=2,
                                      space="PSUM") as ph0_pool:
                        for ci in range(CT):
                            for g in range(2):
                                ptg = ph0_pool.tile([128, 4, 128], f16,
                                                    tag="ptg",
                                                    name=f"ptg{ci}_{g}")
                                for k in range(4):
                                    ni = g * 4 + k
                                    nc.tensor.transpose(
                                        ptg[:, k, :],
                                        xt_ins[ni][:,
                                                   ci * 128:(ci + 1) * 128],
                                        ident[:])
                                cp = (nc.vector.tensor_copy
                                      if (ci * 2 + g) % 2 else nc.scalar.copy)
                                cp(xT[ci][:, g * 512:(g + 1) * 512],
                                   ptg[:].rearrange("p a b -> p (a b)"))
                            for k in range(3):
                                nc.tensor.matmul(wps[:], ident[:], ident[:],
                                                 start=(k == 0), stop=(k == 2))
                    wrm_cm.__exit__(None, None, None)
                    acc_cm = tc.tile_pool(name="acc", bufs=2, space="PSUM")
                    acc_pool = acc_cm.__enter__()

                    def qk_proj(fi):
                        pqk = [acc_pool.tile([128, 512], f32, tag="acc",
                                             name=f"pqk{fi}_{ch}")
                               for ch in range(NCH)]
                        for ci in range(CT):
                            for ch in range(NCH):
                                nc.tensor.matmul(
                                    pqk[ch][:],
                                    wq[ci][:, fi * 128:(fi + 1) * 128],
                                    xT[ci][:, ch * 512:(ch + 1) * 512],
                                    start=(ci == 0), stop=(ci == CT - 1))
                        for ch in range(NCH):
                            nc.vector.tensor_copy(
                                qkT[fi][:, ch * 512:(ch + 1) * 512],
                                pqk[ch][:])

                    def v_proj(ni):
                        pv = [acc_pool.tile([128, VCH], f32, tag="acc",
                                            name=f"pv{ni}_{vc}")
                              for vc in range(2)]
                        for ci in range(CT):
                            for vc in range(2):
                                nc.tensor.matmul(
                                    pv[vc][:],
                                    xT[ci][:, ni * 128:(ni + 1) * 128],
                                    wq[ci][:, FQK + vc * VCH:
                                           FQK + (vc + 1) * VCH],
                                    start=(ci == 0), stop=(ci == CT - 1))
                        for vc in range(2):
                            nc.vector.tensor_copy(
                                vnat[:, ni, vc * 6:(vc + 1) * 6, 0:D],
                                pv[vc][:].rearrange("p (h d) -> p h d", d=D))

                    qk_proj(0)
                    qk_proj(6)
                    nc.vector.tensor_copy(bias16[:], bias_sb[:])

                    # ---- attention, head pairs, qk for pair j+1 interleaved
                    attn_pools = (
                        tc.tile_pool(name="fin", bufs=1),
                        tc.tile_pool(name="rc", bufs=2),
                        tc.tile_pool(name="exp", bufs=15),
                        tc.tile_pool(name="pss", bufs=2, space="PSUM"),
                        tc.tile_pool(name="pso", bufs=2, space="PSUM"),
                    )
                    fin_pool, rc_pool, exp_pool, pss_pool, pso_pool = [
                        p.__enter__() for p in attn_pools]

                    def scores_exp(j, exp_t):
                        pair = (2 * j, 2 * j + 1)
                        for mi in range(NT):
                            ps = {}
                            for h in pair:
                                ps[h] = pss_pool.tile([128, N], f32, tag="pss",
                                                      name=f"pss{h}_{mi}")
                            # adjacent e/o matmuls -> disjoint PE row groups
                            for ch in range(NCH):
                                for h in pair:
                                    qrow = (h % 2) * D
                                    nc.tensor.matmul(
                                        ps[h][:, ch * 512:(ch + 1) * 512],
                                        qkT[6 + h // 2][qrow:qrow + D,
                                                        mi * 128:(mi + 1) * 128],
                                        qkT[h // 2][qrow:qrow + D,
                                                    ch * 512:(ch + 1) * 512],
                                        start=True, stop=True)
                            for h in pair:
                                if mi % 2 == 0:
                                    et = exp_pool.tile([128, 2, N], f16,
                                                       tag="exp",
                                                       name=f"exp{h}_{mi // 2}")
                                    exp_t[h].append(et)
                                nc.scalar.activation(
                                    exp_t[h][mi // 2][:, mi % 2, :], ps[h][:],
                                    mybir.ActivationFunctionType.Exp,
                                    bias=exp_bias[:], scale=SCALE)

                    def attnv_norm(j, exp_t, po_pool, use_act):
                        pair = (2 * j, 2 * j + 1)
                        for ch in range(NCH):
                            po = {}
                            for h in pair:
                                po[h] = po_pool.tile(
                                    [D + 1, 512], f32, tag="po",
                                    name=f"po{h}_{ch}")
                            for mi in range(NT):
                                for h in pair:
                                    nc.tensor.matmul(
                                        po[h][:],
                                        vnat[:, mi, h, :],
                                        exp_t[h][mi // 2][:, mi % 2,
                                                          ch * 512:
                                                          (ch + 1) * 512],
                                        start=(mi == 0), stop=(mi == NT - 1))
                            rc = {}
                            ou = {}
                            lateprio = tc.high_priority(offset=-60)
                            lateprio.__enter__()
                            for h in pair:
                                rs = rc_pool.tile([1, 512], f32, tag="rs",
                                                  name=f"rs{h}_{ch}", bufs=2)
                                (nc.scalar.copy if use_act
                                 else nc.vector.tensor_copy)(
                                    rs[:], po[h][D:D + 1, :])
                                rcf = rc_pool.tile([1, 512], f32, tag="rcf",
                                                   name=f"rcf{h}_{ch}", bufs=2)
                                nc.vector.reciprocal_approx_fast(rcf[:], rs[:])
                                rc[h] = rc_pool.tile([1, 512], f16, tag="rc",
                                                     name=f"rc{h}_{ch}",
                                                     bufs=2)
                                (nc.scalar.copy if use_act
                                 else nc.vector.tensor_copy)(rc[h][:], rcf[:])
                                ou[h] = rc_pool.tile([D, 512], f16, tag="ou",
                                                     name=f"ou{h}_{ch}",
                                                     bufs=2)
                                if use_act:
                                    nc.scalar.copy(ou[h][:], po[h][0:D, :])
                                else:
                                    nc.vector.tensor_copy(ou[h][:],
                                                          po[h][0:D, :])
                            for h in pair:
                                # per-head broadcast of 1/den to 64 rows
                                pb = po_pool.tile([D, 512], f32, tag="po",
                                                  name=f"pb{h}_{ch}")
                                nc.tensor.matmul(
                                    pb[:], sel[0:1, 0:D], rc[h][:],
                                    start=True, stop=True)
                                s = h // 2
                                nc.vector.tensor_mul(
                                    onorm[64 * (h % 2):64 * (h % 2) + D,
                                          s // 2, s % 2,
                                          ch * 512:(ch + 1) * 512],
                                    ou[h][:], pb[0:D, :])
                            lateprio.__exit__(None, None, None)

                    finA = {}

                    def proj_a():
                        # head pairs 0-3 (cpairs 0-1) + bias via an identity
                        # matmul accumulate; keep fp16 partials in SBUF.
                        # Late scheduler priority: nothing reads finA until
                        # the tail, and hoisting these matmuls ahead of the
                        # last pairs' scores starves the exp pipeline
                        lp = tc.high_priority(offset=-80)
                        lp.__enter__()
                        for ni in range(NT):
                            pf = [acc_pool.tile([128, VCH], f32, tag="acc",
                                                name=f"pfa{ni}_{fc}")
                                  for fc in range(2)]
                            for cp in range(2):
                                for jj in range(2):
                                    for fc in range(2):
                                        nc.tensor.matmul(
                                            pf[fc][:],
                                            onorm[:, cp, jj,
                                                  ni * 128:(ni + 1) * 128],
                                            wp[cp * 2 + jj][:,
                                                fc * VCH:(fc + 1) * VCH],
                                            start=(cp == 0 and jj == 0),
                                            stop=False)
                            for fc in range(2):
                                nc.tensor.matmul(
                                    pf[fc][:], ident[:],
                                    bias16[:, fc * VCH:(fc + 1) * VCH],
                                    start=False, stop=True)
                            fa = fin_pool.tile([128, C], f16, tag="finA",
                                               name=f"finA{ni}", bufs=8)
                            finA[ni] = fa
                            for fc in range(2):
                                sl = slice(fc * VCH, (fc + 1) * VCH)
                                nc.vector.tensor_copy(fa[:, sl], pf[fc][:])
                        lp.__exit__(None, None, None)

                    def proj_b(ni, po_pool):
                        # heads 8-11 (cp2) + accumulated fp16 partial in one
                        # pass; PSUM->SBUF hops split across Scalar and DVE
                        pf = [po_pool.tile([128, VCH], f32, tag="po",
                                           name=f"pfb{ni}_{fc}")
                              for fc in range(2)]
                        for fc in range(2):
                            for jj in range(2):
                                nc.tensor.matmul(
                                    pf[fc][:],
                                    onorm[:, 2, jj, ni * 128:(ni + 1) * 128],
                                    wp[4 + jj][:, fc * VCH:(fc + 1) * VCH],
                                    start=(jj == 0), stop=False)
                            nc.tensor.matmul(
                                pf[fc][:], ident[:],
                                finA[ni][:, fc * VCH:(fc + 1) * VCH],
                                start=False, stop=True)
                        fin = fin_pool.tile([128, C], f32, tag="fin",
                                            name=f"fin{ni}", bufs=3)
                        nc.scalar.copy(fin[:, 0:VCH], pf[0][:])
                        nc.vector.tensor_copy(fin[:, VCH:C], pf[1][:])
                        eng = nc.sync if ni % 2 == 0 else nc.gpsimd
                        eng.dma_start(
                            out_d[ni * 128:(ni + 1) * 128, :], fin[:])

                    exp_ts = {}
                    for j in range(H // 2):
                        exp_ts[j] = {2 * j: [], 2 * j + 1: []}
                        scores_exp(j, exp_ts[j])
                        if j + 1 < H // 2:
                            qk_proj(j + 1)
                            qk_proj(6 + j + 1)
                        if j == 0:
                            nc.vector.tensor_copy(
                                vnat[:, :, :, D].rearrange(
                                    "p a h -> p (a h)"),
                                vones_f32[:])
                            for ni in range(NT):
                                v_proj(ni)
                        if j >= 1:
                            attnv_norm(j - 1, exp_ts.pop(j - 1), pso_pool,
                                       use_act=False)
                        if j == 4:
                            proj_a()
                    # last pair: scores PSUM banks are free now; run its
                    # attn@v from a fresh 4-slot pool so it never waits on
                    # the norm chain's pb slot recycling
                    attn_pools[4].__exit__(None, None, None)
                    attn_pools[3].__exit__(None, None, None)
                    with tc.tile_pool(name="tail", bufs=6,
                                      space="PSUM") as tail_pool:
                        # last pair: both attn@v chunk chains dense first,
                        # then per chunk: norm, then the proj rows that only
                        # need that chunk's columns (overlaps the other norm)
                        pair = (H - 2, H - 1)
                        exp_t = exp_ts.pop(H // 2 - 1)
                        po = {}
                        for ch in range(NCH):
                            for h in pair:
                                po[h, ch] = tail_pool.tile(
                                    [D + 1, 512], f32, tag="po",
                                    name=f"tpo{h}_{ch}")
                            for mi in range(NT):
                                for h in pair:
                                    nc.tensor.matmul(
                                        po[h, ch][:],
                                        vnat[:, mi, h, :],
                                        exp_t[h][mi // 2][:, mi % 2,
                                                          ch * 512:
                                                          (ch + 1) * 512],
                                        start=(mi == 0), stop=(mi == NT - 1))
                        for ch in range(NCH):
                            rc = {}
                            ou = {}
                            for h in pair:
                                rs = rc_pool.tile([1, 512], f32, tag="rs",
                                                  name=f"trs{h}_{ch}", bufs=2)
                                nc.scalar.copy(rs[:], po[h, ch][D:D + 1, :])
                                rcf = rc_pool.tile([1, 512], f32, tag="rcf",
                                                   name=f"trcf{h}_{ch}",
                                                   bufs=2)
                                nc.vector.reciprocal_approx_fast(rcf[:],
                                                                 rs[:])
                                rc[h] = rc_pool.tile([1, 512], f16, tag="rc",
                                                     name=f"trc{h}_{ch}",
                                                     bufs=2)
                                nc.scalar.copy(rc[h][:], rcf[:])
                                ou[h] = rc_pool.tile([D, 512], f16, tag="ou",
                                                     name=f"tou{h}_{ch}",
                                                     bufs=2)
                                nc.scalar.copy(ou[h][:], po[h, ch][0:D, :])
                            for h in pair:
                                pb = tail_pool.tile([D, 512], f32, tag="po",
                                                    name=f"tpb{h}_{ch}")
                                nc.tensor.matmul(
                                    pb[:], sel[0:1, 0:D], rc[h][:],
                                    start=True, stop=True)
                                s = h // 2
                                nc.vector.tensor_mul(
                                    onorm[64 * (h % 2):64 * (h % 2) + D,
                                          s // 2, s % 2,
                                          ch * 512:(ch + 1) * 512],
                                    ou[h][:], pb[0:D, :])
                            for ni in range(ch * 4, ch * 4 + 4):
                                proj_b(ni, tail_pool)

                    for p in (attn_pools[2], attn_pools[1], attn_pools[0]):
                        p.__exit__(None, None, None)
                    acc_cm.__exit__(None, None, None)

    nc.compile()
    return nc


def _get_compiled():
    global _compiled
    if _compiled is None:
        _compiled = _build()
    return _compiled


def _run(x, w_qkv, w_proj, b_proj, **kwargs):
    from concourse.bass_utils import run_bass_kernel_spmd

    x = np.asarray(x, dtype=np.float32).astype(np.float16)
    w_qkv = np.ascontiguousarray(
        np.asarray(w_qkv, dtype=np.float32).astype(np.float16))
    w_proj = np.ascontiguousarray(
        np.asarray(w_proj, dtype=np.float32).astype(np.float16))
    b_bcast = np.ascontiguousarray(
        np.broadcast_to(np.asarray(b_proj, dtype=np.float32), (128, C)))

    nc = _get_compiled()
    in_maps = [
        {"x": np.ascontiguousarray(x[b]), "w_qkv": w_qkv,
         "w_proj": w_proj, "b_bcast": b_bcast}
        for b in range(B)
    ]
    return run_bass_kernel_spmd(nc, in_maps, core_ids=list(range(B)), **kwargs)


def kernel(x, w_qkv, w_proj, b_proj, **_):
    res = _run(x, w_qkv, w_proj, b_proj)
    return np.stack([res.results[b]["out"] for b in range(B)], axis=0)
